# revision 2
# baseline (speedup 1.0000x reference)
"""CTC loss (mean, zero_infinity) on 8 TRN2 NeuronCores — 4-step-fold version.

Data-parallel over batch: 4 samples/core. Per core, a prob-domain CTC forward
DP in the rotated label coordinate system (answer at partition 127), with FOUR
time steps folded per serial iteration:

  - gather one-hot matmul -> PL/PB bf16 prob tiles (128, T, NB); QL = PL*skip
  - closed-form band-3 "2-step operator" coefficients, column-major diagonals
    C[j, src, dst] per chunk, built with full-width bf16 DVE passes; the
    [c+1]/[c+2]-row factors come from DMA partition-shifted prob copies
  - pairs of 2-step operators are merged into 4-step band-5 operators
    (banded composition; partition shifts via DMA, zero tails via Pool
    memset); tail merge passes stream into the serial loop's DVE idle gaps
  - serial loop (256 iters): Y = C[chunk] * bcast(state_psum)  (one DVE TT),
    then 9 tiny PE shift-matmuls accumulate next state into PSUM
  - range: host bakes 2^21 per chunk into the 2nd step's log-probs where not
    absorbed; lazy colsum rescale every RS iters (factors recorded, undone on
    host in f64)
"""

import numpy as np

import concourse.bass as bass
import concourse.bacc as bacc
import concourse.tile as tile
from concourse import mybir
from concourse.bass_utils import run_bass_kernel_spmd

F32 = mybir.dt.float32
BF16 = mybir.dt.bfloat16
I32 = mybir.dt.int32
AF = mybir.ActivationFunctionType
OP = mybir.AluOpType

T = 1024
V = 512
L = 128
NB = 4
NCORES = 8
NCH = (T - 2) // 2          # 511 two-step chunks (t = 2+2c, 3+2c)
NIT = NCH + 1               # serial iterations (chunk 0 = warmup t=1)
RS = 8                      # rescale every RS serial iterations
NRS = (NIT // 2 - 1) // RS  # number of applied rescales
BAKE = 21                   # 2^BAKE baked into step-2 slices (host side)
NEG = -1e30


def build_nc(debug_dump=False):
    nc = bacc.Bacc("TRN2", target_bir_lowering=False, debug=False,
                   num_devices=NCORES)

    lpT = nc.dram_tensor("lpT", [NB, V, T], BF16, kind="ExternalInput")
    lpb = nc.dram_tensor("lpb", [NB, T], F32, kind="ExternalInput")   # baked
    thr = nc.dram_tensor("thr", [NB, T], F32, kind="ExternalInput")   # baked
    uneg = nc.dram_tensor("uneg", [NB, L], F32, kind="ExternalInput")
    tgtrot = nc.dram_tensor("tgtrot", [NB, L], F32, kind="ExternalInput")
    skiprot = nc.dram_tensor("skiprot", [L, NB], F32, kind="ExternalInput")
    initm = nc.dram_tensor("initm", [L, NB], F32, kind="ExternalInput")
    outd = nc.dram_tensor("out", [128, 2 * NB], F32, kind="ExternalOutput")
    outs = nc.dram_tensor("scales", [1, NRS * NB], F32, kind="ExternalOutput")
    if debug_dump:
        dbgC = nc.dram_tensor("dbgC", [128, 4 * 48], F32, kind="ExternalOutput")
        dbgS = nc.dram_tensor("dbgS", [128, 8 * 8], F32, kind="ExternalOutput")

    with tile.TileContext(nc) as tc:
        with tc.tile_pool(name="const", bufs=1) as const, \
             tc.tile_pool(name="bigp", bufs=1) as bigp:

            # ---------- constants ----------
            ones_row = const.tile([1, 128], F32)
            nc.vector.memset(ones_row, 1.0)
            ones_col = const.tile([128, 1], F32)
            nc.vector.memset(ones_col, 1.0)
            ones_colb = const.tile([128, 1], BF16)
            nc.vector.memset(ones_colb, 1.0)
            one_row_t = const.tile([1, 512], F32)
            nc.vector.memset(one_row_t, 1.0)

            io_f_i = const.tile([128, 128], I32)
            nc.gpsimd.iota(io_f_i, pattern=[[1, 128]], base=0,
                           channel_multiplier=0)
            io_p_i = const.tile([128, 128], I32)
            nc.gpsimd.iota(io_p_i, pattern=[[0, 128]], base=0,
                           channel_multiplier=1)
            dmk = const.tile([128, 128], F32)
            io_f = const.tile([128, 128], F32)
            nc.vector.tensor_copy(io_f, io_f_i)
            io_p = const.tile([128, 128], F32)
            nc.vector.tensor_copy(io_p, io_p_i)
            nc.vector.tensor_sub(dmk, io_f, io_p)     # free - partition
            SJ = []                                    # shift weights bf16
            for j in range(5):
                s = const.tile([128, 128], BF16, tag=f"S{j}")
                nc.vector.tensor_scalar(s, dmk, float(j), None, OP.is_equal)
                SJ.append(s)

            iota_k = []
            for vc in range(4):
                ik_i = const.tile([128, 1], I32, tag=f"ik{vc}i")
                nc.gpsimd.iota(ik_i, pattern=[[0, 1]], base=128 * vc,
                               channel_multiplier=1)
                ik = const.tile([128, 1], F32, tag=f"ik{vc}")
                nc.vector.tensor_copy(ik, ik_i)
                iota_k.append(ik)

            # ---------- small input loads ----------
            probs_pool = tc.tile_pool(name="probs", bufs=1)
            probs = probs_pool.__enter__()
            lpb_b, thr_b, tgt_b, uneg_b = [], [], [], []
            for b in range(NB):
                tb = probs.tile([1, T], F32, tag=f"lpb{b}")
                nc.sync.dma_start(out=tb, in_=lpb[b:b + 1, :])
                lpb_b.append(tb)
                tb = probs.tile([1, T], F32, tag=f"thr{b}")
                nc.sync.dma_start(out=tb, in_=thr[b:b + 1, :])
                thr_b.append(tb)
                tb = const.tile([1, L], F32, tag=f"tgt{b}")
                nc.sync.dma_start(out=tb, in_=tgtrot[b:b + 1, :])
                tgt_b.append(tb)
                tb = const.tile([1, L], F32, tag=f"un{b}")
                nc.sync.dma_start(out=tb, in_=uneg[b:b + 1, :])
                uneg_b.append(tb)
            skpS = const.tile([L, NB], F32)
            nc.sync.dma_start(out=skpS, in_=skiprot[:, :])
            initS = const.tile([L, NB], F32)
            nc.sync.dma_start(out=initS, in_=initm[:, :])

            # ---------- probability tiles (bf16, scoped) ----------
            PL = probs.tile([128, T, NB], BF16)
            PB = probs.tile([128, T, NB], BF16)
            QL = probs.tile([128, T, NB], BF16)

            TC = 512
            n_tc = T // TC
            with tc.tile_pool(name="psg", bufs=2, space="PSUM") as psg, \
                 tc.tile_pool(name="psT", bufs=2, space="PSUM") as psT, \
                 tc.tile_pool(name="stage", bufs=2) as stage, \
                 tc.tile_pool(name="ohp", bufs=2) as ohp:
                for b in range(NB):
                    pT = psT.tile([128, L], F32, tag="pT")
                    nc.tensor.matmul(pT, ones_row, tgt_b[b], start=True,
                                     stop=True)
                    ohs = []
                    for vc in range(4):
                        oh = ohp.tile([128, L], BF16, tag=f"oh{vc}")
                        nc.vector.tensor_scalar(oh, pT, iota_k[vc], None,
                                                OP.is_equal, OP.bypass)
                        ohs.append(oh)
                    sts = []
                    for vc in range(4):
                        st = stage.tile([128, T], BF16, tag=f"st{vc}")
                        eng = [nc.sync, nc.gpsimd, nc.scalar, nc.sync][vc]
                        eng.dma_start(
                            out=st, in_=lpT[b, 128 * vc:128 * (vc + 1), :])
                        sts.append(st)
                    for tci in range(n_tc):
                        pg = psg.tile([128, TC], F32, tag="pg")
                        for vc in range(4):
                            nc.tensor.matmul(pg, ohs[vc],
                                             sts[vc][:, TC * tci:TC * (tci + 1)],
                                             start=(vc == 0), stop=False)
                        # + thr (absorb + bake), broadcast over partitions
                        nc.tensor.matmul(pg, ones_row,
                                         thr_b[b][:, TC * tci:TC * (tci + 1)],
                                         start=False, stop=False)
                        # + uneg (unused label slots), broadcast over t
                        nc.tensor.matmul(pg, uneg_b[b],
                                         one_row_t[:, 0:TC],
                                         start=False, stop=True)
                        nc.scalar.activation(PL[:, TC * tci:TC * (tci + 1), b],
                                             pg, AF.Exp)
                        pgb = psg.tile([128, TC], F32, tag="pg")
                        nc.tensor.matmul(pgb, ones_row,
                                         lpb_b[b][:, TC * tci:TC * (tci + 1)],
                                         start=True, stop=True)
                        nc.scalar.activation(PB[:, TC * tci:TC * (tci + 1), b],
                                             pgb, AF.Exp)
            # QL = PL * skip   (skip bcast over t)
            skb = const.tile([L, NB], BF16)
            nc.vector.tensor_copy(skb, skpS)
            nc.vector.tensor_tensor(
                QL, PL, skb.unsqueeze(1).broadcast_to([128, T, NB]), OP.mult)

            # ---------- shifted prob copies (partition shift via DMA) ----
            # x2s1[c] = X[c+1, t2(c)], etc.  t1(c)=2+2c, t2(c)=3+2c
            cb_pool = tc.tile_pool(name="cbuild", bufs=1)
            cbuild = cb_pool.__enter__()
            zrow = cbuild.tile([2, NCH, NB], BF16)
            nc.vector.memset(zrow, 0.0)
            pl2s1 = cbuild.tile([128, NCH, NB], BF16)
            ql1s1 = cbuild.tile([128, NCH, NB], BF16)
            ql2s1 = cbuild.tile([128, NCH, NB], BF16)
            ql2s2 = cbuild.tile([128, NCH, NB], BF16)
            # contiguous staging (DVE strided reads are free; DMA needs
            # contiguous rows to avoid descriptor explosion)
            pl2c = cbuild.tile([128, NCH, NB], BF16)
            ql1c = cbuild.tile([128, NCH, NB], BF16)
            ql2c = cbuild.tile([128, NCH, NB], BF16)
            nc.vector.tensor_copy(pl2c, PL[:, 3:T:2, :])
            nc.vector.tensor_copy(ql1c, QL[:, 2:T:2, :])
            nc.vector.tensor_copy(ql2c, QL[:, 3:T:2, :])
            nc.sync.dma_start(out=pl2s1[0:127], in_=pl2c[1:128])
            nc.sync.dma_start(out=pl2s1[127:128], in_=zrow[0:1])
            nc.sync.dma_start(out=ql1s1[0:127], in_=ql1c[1:128])
            nc.sync.dma_start(out=ql1s1[127:128], in_=zrow[0:1])
            nc.sync.dma_start(out=ql2s1[0:127], in_=ql2c[1:128])
            nc.sync.dma_start(out=ql2s1[127:128], in_=zrow[0:1])
            nc.sync.dma_start(out=ql2s2[0:126], in_=ql2c[2:128])
            nc.sync.dma_start(out=ql2s2[126:128], in_=zrow[0:2])

            # ---------- 2-step coefficients ----------
            # SER (128, NIT, 3, 2, 2, NB): [iter][j][src][dst][b]
            # layout: [p, iter, src, j*2+dst, b]
            SER = bigp.tile([128, NIT, 2, 6, NB], BF16)
            # zero only the always-zero slots + warmup-only-zero slots
            nc.vector.memset(SER[:, :, 1, 0, :], 0.0)      # bl0
            nc.vector.memset(SER[:, :, 0, 4, :], 0.0)      # bb2
            nc.vector.memset(SER[:, :, 0, 5, :], 0.0)      # lb2
            nc.vector.memset(SER[:, 0], 0.0)               # warmup full
            C = SER[:, 1:NIT]            # chunks 1..511  (c = iter-1)
            pb1 = PB[:, 2:T:2, :]
            pb2 = PB[:, 3:T:2, :]
            pl1 = PL[:, 2:T:2, :]
            pl2 = PL[:, 3:T:2, :]

            def cs(j, s, d):
                return C[:, :, s, 2 * j + d, :]

            with tc.tile_pool(name="ctmp", bufs=1) as ctmp:
                # dst B (d=0):
                nc.vector.tensor_tensor(cs(0, 0, 0), pb2, pb1, OP.mult)  # Cbb0
                nc.vector.tensor_tensor(cs(1, 0, 0), pb2, pl1, OP.mult)  # Cbb1
                nc.vector.tensor_tensor(cs(1, 1, 0), cs(0, 0, 0),
                                        cs(1, 0, 0), OP.add)             # Cbl1
                nc.vector.tensor_tensor(cs(2, 1, 0), pb2, ql1s1, OP.mult)  # Cbl2
                # dst L (d=1):
                t1 = ctmp.tile([128, NCH, NB], BF16, tag="t1")
                nc.vector.tensor_tensor(t1, pl1, pb1, OP.add)
                nc.vector.tensor_tensor(cs(0, 0, 1), pl2, t1, OP.mult)   # Clb0
                nc.vector.tensor_tensor(cs(0, 1, 1), pl2, pl1, OP.mult)  # Cll0
                nc.vector.tensor_tensor(cs(1, 0, 1), ql2s1, pl1, OP.mult)  # Clb1
                t2 = ctmp.tile([128, NCH, NB], BF16, tag="t2")
                nc.vector.tensor_tensor(t2, ql1s1, pb1, OP.add)
                t3 = ctmp.tile([128, NCH, NB], BF16, tag="t3")
                nc.vector.tensor_tensor(t3, pl2s1, t2, OP.mult)
                nc.vector.tensor_tensor(cs(1, 1, 1), t3, cs(1, 0, 1),
                                        OP.add)                          # Cll1
                nc.vector.tensor_tensor(cs(2, 1, 1), ql2s2, ql1s1,
                                        OP.mult)                         # Cll2

                # warmup chunk (iter 0): 1-step operator at t=1
                W = SER[:, 0]
                ql1w = ctmp.tile([128, 1, NB], BF16, tag="qw")
                nc.vector.memset(ql1w, 0.0)
                nc.sync.dma_start(out=ql1w[0:127], in_=QL[1:128, 1:2, :])
                nc.vector.tensor_copy(W[:, 0, 0, :], PB[:, 1, :])   # bb0=pb1
                nc.vector.tensor_copy(W[:, 1, 2, :], PB[:, 1, :])   # bl1=pb1
                nc.vector.tensor_copy(W[:, 0, 1, :], PL[:, 1, :])   # lb0=pl1
                nc.vector.tensor_copy(W[:, 1, 1, :], PL[:, 1, :])   # ll0=pl1
                nc.vector.tensor_copy(W[:, 1, 3, :], ql1w[:, 0, :])  # ll1

            initBL = const.tile([128, 2, NB], F32)
            nc.vector.tensor_tensor(initBL[:, 0, :], initS, PB[:, 0, :],
                                    OP.mult)
            nc.vector.tensor_tensor(initBL[:, 1, :], initS, PL[:, 0, :],
                                    OP.mult)
            cb_pool.__exit__(None, None, None)
            probs_pool.__exit__(None, None, None)

            # ---------- merge pairs: 2-step ops -> 4-step band-5 ops ----
            NI2 = NIT // 2
            SER4 = bigp.tile([128, NI2, 2, 10, NB], BF16)
            mrg_pool = tc.tile_pool(name="mrg", bufs=1)
            mrg = mrg_pool.__enter__()
            mprod_pool = tc.tile_pool(name="mprod", bufs=2)
            mprod = mprod_pool.__enter__()
            Bct = mrg.tile([128, NI2, 2, 6, NB], BF16)
            nc.vector.tensor_copy(Bct, SER[:, 1:NIT:2])
            Bs1 = mrg.tile([128, NI2, 2, 6, NB], BF16)
            Bs2 = mrg.tile([128, NI2, 2, 6, NB], BF16)
            nc.gpsimd.memset(Bs1, 0.0)
            nc.gpsimd.memset(Bs2, 0.0)
            nc.sync.dma_start(out=Bs1[0:127], in_=Bct[1:128])
            nc.scalar.dma_start(out=Bs2[0:126], in_=Bct[2:128])
            nc.vector.memset(SER4[:, :, :, 6:10, :], 0.0)
            Bq = [Bct, Bs1, Bs2]
            Aodd = SER[:, 0:NIT:2]        # (128, NI2, 2, 6, NB)

            # A zero slots: (s=0, j2=2, mid=*) = bb2/lb2; (s=1, j2=0,
            # mid=0) = bl0.  First writer per (s, window) does a plain
            # mult; later combos accumulate.  Emitted per pair-range so the
            # tail ranges interleave into the serial loop (DVE idle gaps).
            def emit_merge_range(p0, p1):
                n = p1 - p0
                for j2 in range(3):
                    for mid in range(2):
                        for s in range(2):
                            if s == 0 and j2 == 2:
                                continue
                            if s == 1 and j2 == 0 and mid == 0:
                                continue
                            asl = Aodd[:, p0:p1, s, 2 * j2 + mid, :]
                            ab = asl.unsqueeze(2).broadcast_to(
                                [128, n, 6, NB])
                            bsl = Bq[j2][:, p0:p1, mid, :, :]
                            osl = SER4[:, p0:p1, s, 2 * j2:2 * j2 + 6, :]
                            first = (j2 == 0 and
                                     ((s == 0 and mid == 0) or
                                      (s == 1 and mid == 1)))
                            if first:
                                yield nc.vector.tensor_tensor(osl, bsl, ab,
                                                              OP.mult)
                            else:
                                pr = mprod.tile([128, n, 6, NB], BF16,
                                                tag="pr")
                                yield nc.vector.tensor_tensor(pr, bsl, ab,
                                                              OP.mult)
                                yield nc.vector.tensor_tensor(osl, osl, pr,
                                                              OP.add)

            MSTRIP = 8
            # pairs [0:32) merged up-front; the rest stream into the loop
            for p0 in range(0, 16, MSTRIP):
                for _ in emit_merge_range(p0, p0 + MSTRIP):
                    pass
            pending = []
            for p0 in range(16, NI2, MSTRIP):
                pending.append(emit_merge_range(p0, p0 + MSTRIP))
            pending.reverse()

            # ---------- serial loop ----------
            logS = const.tile([1, NRS, NB], F32)
            nc.vector.memset(logS, 1.0)

            with tc.tile_pool(name="pstep", bufs=4, space="PSUM") as pstep, \
                 tc.tile_pool(name="psr", bufs=2, space="PSUM") as psr, \
                 tc.tile_pool(name="work", bufs=4) as work:

                # init state in psum: ps[:, 0, :] = initm*PB[:,0,:],
                #                     ps[:, 1, :] = initm*PL[:,0,:]
                ps = pstep.tile([128, 2, NB], F32, tag="ps")
                nc.vector.tensor_copy(ps, initBL)

                scP = work.tile([128, NB], F32, tag="scP")    # pending scale
                have_scale = False
                if debug_dump:
                    sd = const.tile([128, 8 * 8], F32)

                YENG = nc.vector
                NI2 = NIT // 2
                pairs = [(j, s) for j in range(5) for s in range(2)
                         if not (j == 4 and s == 0)]

                def drain_merge(k):
                    while k > 0 and pending:
                        try:
                            next(pending[-1])
                            k -= 1
                        except StopIteration:
                            pending.pop()

                for it in range(NI2):
                    drain_merge(3)
                    Y = work.tile([128, 2, 10, NB], BF16, tag="Y")
                    sb = ps.unsqueeze(2).broadcast_to([128, 2, 10, NB])
                    YENG.tensor_tensor(Y, SER4[:, it], sb, OP.mult)
                    if have_scale:
                        scb = scP.unsqueeze(1).unsqueeze(1) \
                            .broadcast_to([128, 2, 10, NB])
                        Y2 = work.tile([128, 2, 10, NB], BF16, tag="Yb")
                        YENG.tensor_tensor(Y2, Y, scb, OP.mult)
                        Y = Y2
                        have_scale = False
                    psn = pstep.tile([128, 2, NB], F32, tag="ps")
                    for n, (j, s) in enumerate(pairs):
                        nc.tensor.matmul(psn, SJ[j],
                                         Y[:, s, 2 * j:2 * j + 2, :],
                                         start=(n == 0),
                                         stop=(n == len(pairs) - 1))
                    ps = psn

                    if it % RS == RS - 1 and it < NI2 - 1:
                        ri = it // RS
                        # colsum of Y (proxy for state mass) -> scale
                        pss = psr.tile([1, NB], F32, tag="pss")
                        fl = Y.rearrange("p a b c -> p (a b c)")
                        for g in range(20):
                            nc.tensor.matmul(pss, ones_colb,
                                             fl[:, 4 * g:4 * (g + 1)],
                                             start=(g == 0), stop=(g == 19))
                        nc.scalar.copy(logS[:, ri, :], pss)
                        srec = work.tile([1, NB], F32, tag="srec")
                        nc.vector.reciprocal(srec, pss)
                        # broadcast to 128 partitions via PE
                        psb = psr.tile([128, NB], F32, tag="psb")
                        nc.tensor.matmul(psb, ones_row, srec, start=True,
                                         stop=True)
                        nc.vector.tensor_copy(scP, psb)
                        have_scale = True

                    if debug_dump and it < 8:
                        nc.vector.tensor_copy(
                            sd[:, 8 * it:8 * (it + 1)],
                            ps.rearrange("p a b -> p (a b)"))

                if debug_dump:
                    nc.sync.dma_start(out=dbgS[:, :], in_=sd)

                # ---------- output ----------
                fin = work.tile([128, 2 * NB], F32, tag="fin")
                nc.vector.tensor_copy(fin, ps.rearrange("p a b -> p (a b)"))
                nc.sync.dma_start(out=outd[:, :], in_=fin)
                nc.sync.dma_start(
                    out=outs[:, :],
                    in_=logS.rearrange("p a b -> p (a b)"))

            mprod_pool.__exit__(None, None, None)
            mrg_pool.__exit__(None, None, None)

    nc.compile()
    return nc


def host_prep(log_probs, targets, input_lengths, target_lengths):
    import jax.numpy as jnp
    log_probs = np.asarray(log_probs, np.float32)
    targets = np.asarray(targets).astype(np.int64)
    il = np.asarray(input_lengths).astype(np.int64)
    tl = np.asarray(target_lengths).astype(np.int64)
    t_ar = np.arange(T)
    bake = float(BAKE * np.log(2.0))
    # t2 slice times: t = 3 + 2c
    is_t2 = np.zeros(T, np.float32)
    is_t2[3::2] = 1.0
    in_maps = []
    for c in range(NCORES):
        s = slice(c * NB, (c + 1) * NB)
        lp = log_probs[s]
        ilc, tlc = il[s], tl[s]
        tg = targets[s]
        lpT = np.asarray(jnp.asarray(np.transpose(lp, (0, 2, 1)),
                                     jnp.bfloat16))
        absorb = t_ar[None, :] >= ilc[:, None]
        live = ~absorb
        bk = bake * is_t2[None, :] * live
        thr = np.where(absorb, np.float32(NEG), bk).astype(np.float32)
        lpbm = np.where(absorb, np.float32(0.0),
                        lp[:, :, 0] + bk).astype(np.float32)
        rot = 127 - tlc
        tgtrot = np.full((NB, L), -1.0, np.float32)
        skiprot = np.zeros((L, NB), np.float32)
        unegm = np.full((NB, L), NEG, np.float32)
        initm = np.zeros((L, NB), np.float32)
        for b in range(NB):
            r0 = rot[b]
            n = tlc[b]
            tgtrot[b, r0:r0 + n] = tg[b, :n].astype(np.float32)
            unegm[b, r0:r0 + n] = 0.0
            initm[r0, b] = 1.0
            if n > 1:
                sk = (tg[b, 1:n] != tg[b, :n - 1]).astype(np.float32)
                skiprot[r0 + 1:r0 + n, b] = sk
        in_maps.append({
            "lpT": lpT, "lpb": lpbm, "thr": thr, "uneg": unegm,
            "tgtrot": tgtrot, "skiprot": skiprot, "initm": initm,
        })
    return in_maps


_NC_CACHE = {}


def _get_nc():
    if "nc" not in _NC_CACHE:
        _NC_CACHE["nc"] = build_nc()
    return _NC_CACHE["nc"]


def finish(results, input_lengths, target_lengths):
    il = np.asarray(input_lengths).astype(np.int64)
    tl = np.asarray(target_lengths).astype(np.int64)
    t_ar = np.arange(T)
    is_t2 = np.zeros(T, np.bool_)
    is_t2[3::2] = True
    pers = []
    for c in range(NCORES):
        out = results[c]["out"]          # (128, 2*NB)
        sc = results[c]["scales"][0].astype(np.float64).reshape(NRS, NB)
        ilc = il[c * NB:(c + 1) * NB]
        tlc = tl[c * NB:(c + 1) * NB].astype(np.float64)
        bfin = out[127, 0:NB].astype(np.float64)
        n2 = (is_t2[None, :] & (t_ar[None, :] < ilc[:, None])).sum(1)
        ll = (np.log(np.maximum(bfin, 1e-300))
              + np.log(np.maximum(sc, 1e-300)).sum(0)
              - n2 * BAKE * np.log(2.0))
        per = -ll / tlc
        per = np.where(bfin > 0, per, 0.0)
        pers.append(per)
    return np.float32(np.mean(np.concatenate(pers)))


def kernel(log_probs, targets, input_lengths, target_lengths):
    nc = _get_nc()
    in_maps = host_prep(log_probs, targets, input_lengths, target_lengths)
    res = run_bass_kernel_spmd(nc, in_maps, core_ids=list(range(NCORES)))
    return finish(res.results, input_lengths, target_lengths)
